# revision 2
# baseline (speedup 1.0000x reference)
"""ContextPosSelfAttn (CoPE attention) — fully on-device Trainium2 Bass kernel.

Sharding: leading B (=64) dim split across 8 NeuronCores (8 batches each),
pos_emb replicated.

The data-dependent take_along_axis gather at the heart of CoPE is
restructured using the monotone-staircase property of the positions:
pos[l, m] = sum_{m'=m..l} gates[l, m'] is strictly decreasing in m with
steps < 1, so floor(pos) is a unit-step staircase. The gather of
plf[l, floor(pos[l,m])] therefore changes only at integer crossings, each
integer is crossed at most once per row, and the gathered row equals
  plf[l, 0] + reversed-cumsum over m of (crossing ? D1[l, p*] : 0)
where D1 = diff(plf). Crossing deltas are materialized with GPSIMD
local_scatter (per-partition unique indices), and the reversed cumsums run
on the DVE hardware prefix-scan (tensor_tensor_scan) with reversed access
patterns. Interpolation weight w = frac(pos) and the ceil-side slope use a
second scatter of D2 = diff(D1) through the same crossing map.
"""

import numpy as np

B, L, D = 64, 1024, 64
NPOS = 1025
N_CORES = 8
BPC = B // N_CORES
RT = L // 128          # 8 row tiles
SCALE = 0.125
PAX = NPOS + 1         # 1026, padded p-axis for scatters

_CACHE = {}


def _build_nc():
    import concourse.bacc as bacc
    import concourse.mybir as mybir
    from concourse import tile

    dt = mybir.dt
    Alu = mybir.AluOpType
    Act = mybir.ActivationFunctionType

    nc = bacc.Bacc(None, target_bir_lowering=False, debug=False)

    q_d = nc.dram_tensor("q", [BPC, L, D], dt.float32, kind="ExternalInput")
    k_d = nc.dram_tensor("k", [BPC, L, D], dt.float32, kind="ExternalInput")
    kc_d = nc.dram_tensor("kc", [BPC, L, D], dt.float32, kind="ExternalInput")
    v_d = nc.dram_tensor("v", [BPC, L, D], dt.float32, kind="ExternalInput")
    pe_d = nc.dram_tensor("pe", [D, NPOS], dt.float32, kind="ExternalInput")
    out_d = nc.dram_tensor("out", [BPC, L, D], dt.float16, kind="ExternalOutput")

    with tile.TileContext(nc) as tc:
        with (
            tc.tile_pool(name="const", bufs=1) as cpool,
            tc.tile_pool(name="inp", bufs=2) as ipool,          # per-batch inputs
            tc.tile_pool(name="row", bufs=2) as rpool,          # per-l-tile [128, *]
            tc.tile_pool(name="pax", bufs=2) as ppool,          # p-axis tensors
            tc.tile_pool(name="sm", bufs=3) as spool,           # small tiles
            tc.tile_pool(name="et", bufs=3) as epool,           # transposed exp
            tc.tile_pool(name="ps", bufs=3, space="PSUM") as pbig,     # [128,512]
            tc.tile_pool(name="pst", bufs=2, space="PSUM") as ptr,     # [128,128]
            tc.tile_pool(name="psa", bufs=2, space="PSUM") as pacc,    # [128,65]
        ):
            # ---------------- constants ----------------
            ones128 = cpool.tile([128, 128], dt.float32)
            nc.vector.memset(ones128[:], 1.0)
            ident = cpool.tile([128, 128], dt.float32)
            nc.gpsimd.affine_select(ident[:], ones128[:], [[-1, 128]],
                                    Alu.is_equal, 0.0, base=0,
                                    channel_multiplier=1)
            # tril[l, m] = 1 if m <= l  (valid region of a diagonal tile)
            tril = cpool.tile([128, 128], dt.float32)
            nc.gpsimd.affine_select(tril[:], ones128[:], [[-1, 128]],
                                    Alu.is_ge, 0.0, base=0,
                                    channel_multiplier=1)
            # iota16[:, j] = j + 2 (scatter #1 payload: encoded m)
            iota16 = cpool.tile([128, L], dt.int16)
            nc.gpsimd.iota(iota16[:], [[1, L]], base=2, channel_multiplier=0)
            # pe table [64, 1025]
            pe_sb = cpool.tile([D, NPOS], dt.float32)
            nc.sync.dma_start(pe_sb[:], pe_d[:, :])

            for b in range(BPC):
                # ---------------- load + transpose inputs ----------------
                # qT/kT/kcT: [64, 1024]
                qT = ipool.tile([D, L], dt.float32, tag="qT")
                kT = ipool.tile([D, L], dt.float32, tag="kT")
                kcT = ipool.tile([D, L], dt.float32, tag="kcT")
                for (src, dstT, tg) in ((q_d, qT, "q"), (k_d, kT, "k"),
                                        (kc_d, kcT, "kc")):
                    for t in range(RT):
                        nat = spool.tile([128, D], dt.float32, tag=f"nat{tg}")
                        nc.sync.dma_start(nat[:], src[b, t * 128:(t + 1) * 128, :])
                        tp = ptr.tile([128, 128], dt.float32, tag="tps")
                        nc.tensor.transpose(tp[0:D, :], nat[:], ident[:])
                        nc.scalar.activation(dstT[:, t * 128:(t + 1) * 128],
                                             tp[0:D, :], Act.Copy)
                # vext tiles [128, 65] fp16 with ones column
                vext = []
                for t in range(RT):
                    vt32 = spool.tile([128, D], dt.float32, tag="vt32")
                    nc.sync.dma_start(vt32[:], v_d[b, t * 128:(t + 1) * 128, :])
                    vt = ipool.tile([128, D + 1], dt.float16, tag=f"vext{t}")
                    nc.vector.tensor_copy(vt[:, 0:D], vt32[:])
                    nc.vector.memset(vt[:, D:D + 1], 1.0)
                    vext.append(vt)

                # ---------- phase 1: gates (sigmoid table) + plf/D1/D2 ------
                gts, D1s, D2s, f0s = [], [], [], []
                for t in range(RT):
                    W = 128 * (t + 1)
                    l0 = t * 128
                    qTt = qT[:, l0:l0 + 128]

                    # plf = q @ pe : [128, 1025]
                    plf = ppool.tile([128, NPOS], dt.float32, tag="plf")
                    for c0 in range(0, NPOS, 512):
                        cw = min(512, NPOS - c0)
                        pp = pbig.tile([128, cw], dt.float32, tag="mmps")
                        nc.tensor.matmul(pp[:], qTt, pe_sb[:, c0:c0 + cw],
                                         start=True, stop=True)
                        nc.scalar.activation(plf[:, c0:c0 + cw], pp[:], Act.Copy)
                    # D1[:, j] = plf[:, j] - plf[:, j-1]; col 0 is never read
                    # (its pairing index is always invalid), so left garbage.
                    D1 = ppool.tile([128, W + 1], dt.float16, tag=f"D1_{t}")
                    nc.vector.tensor_tensor(D1[:, 1:W + 1], plf[:, 1:W + 1],
                                            plf[:, 0:W], Alu.subtract)
                    # D2[:, j] = D1[:, j+1] - D1[:, j]; cols 0 / >=1024 unread
                    D2 = ppool.tile([128, W], dt.float16, tag=f"D2_{t}")
                    nc.vector.tensor_tensor(D2[:, 1:W], D1[:, 2:W + 1],
                                            D1[:, 1:W], Alu.subtract)
                    # anchors: col0 = plf[:,0] (F0), col1 = plf[:,1]-plf[:,0]
                    f0 = spool.tile([128, 2], dt.float32, tag=f"f0_{t}")
                    nc.vector.tensor_copy(f0[:, 0:1], plf[:, 0:1])
                    nc.vector.tensor_tensor(f0[:, 1:2], plf[:, 1:2],
                                            plf[:, 0:1], Alu.subtract)
                    D1s.append(D1); D2s.append(D2); f0s.append(f0)

                    # gates
                    g = rpool.tile([128, W], dt.float32, tag=f"g_{t}")
                    for c0 in range(0, W, 512):
                        cw = min(512, W - c0)
                        gp = pbig.tile([128, cw], dt.float32, tag="mmps")
                        nc.tensor.matmul(gp[:], qTt, kcT[:, c0:c0 + cw],
                                         start=True, stop=True)
                        nc.scalar.activation(g[:, c0:c0 + cw], gp[:],
                                             Act.Sigmoid, scale=SCALE)
                    nc.vector.tensor_tensor(g[:, W - 128:W], g[:, W - 128:W],
                                            tril[:], Alu.mult)
                    gts.append(g)

                # ---------- phase 2: positions/scatters/exp (exp table) -----
                for t in range(RT):
                    W = 128 * (t + 1)
                    l0 = t * 128
                    qTt = qT[:, l0:l0 + 128]
                    g, D1, D2, f0 = gts[t], D1s[t], D2s[t], f0s[t]

                    # positions: reversed inclusive cumsum of g
                    pos = rpool.tile([128, L], dt.float32, tag="pos")
                    nc.vector.tensor_tensor_scan(
                        pos[:, 0:W][:, ::-1], g[:, 0:W][:, ::-1],
                        g[:, 0:W][:, ::-1], 0.0, Alu.add, Alu.bypass)

                    # fl = floor(pos) (cast rounds; pos-0.5 rounds to floor)
                    i16p = rpool.tile([128, L], dt.int16, tag="i16p")
                    nc.vector.tensor_scalar(i16p[:, 0:W], pos[:, 0:W], -0.5,
                                            None, Alu.add)
                    fl = rpool.tile([128, L + 1], dt.float16, tag="fl")
                    nc.vector.memset(fl[:, W:W + 1], 0.0)
                    nc.vector.tensor_copy(fl[:, 0:W], i16p[:, 0:W])
                    w = rpool.tile([128, L], dt.float16, tag="w")
                    nc.vector.tensor_tensor(w[:, 0:W], pos[:, 0:W], fl[:, 0:W],
                                            Alu.subtract)
                    # cross[m] = fl[m] - fl[m+1]  (0 or 1, exact)
                    cross = rpool.tile([128, L], dt.float16, tag="cross")
                    nc.vector.tensor_tensor(cross[:, 0:W], fl[:, 0:W],
                                            fl[:, 1:W + 1], Alu.subtract)
                    # idxs1 = cross * (fl + 1) - 1 -> int16 (in-place on cross)
                    nc.vector.scalar_tensor_tensor(cross[:, 0:W], fl[:, 0:W],
                                                   1.0, cross[:, 0:W],
                                                   Alu.add, Alu.mult)
                    idxs1 = rpool.tile([128, L], dt.int16, tag="idxs1")
                    nc.vector.tensor_scalar(idxs1[:, 0:W], cross[:, 0:W], -1.0,
                                            None, Alu.add)

                    # scatter #1: crossing map mu over p-axis
                    mu = ppool.tile([128, PAX], dt.int16, tag="mu")
                    nc.gpsimd.local_scatter(mu[:], iota16[:, 0:W],
                                            idxs1[:, 0:W], channels=128,
                                            num_elems=PAX, num_idxs=W)
                    idxs2 = ppool.tile([128, PAX], dt.int16, tag="idxs2")
                    nc.vector.tensor_scalar(idxs2[:], mu[:], -2, None, Alu.add)

                    # scatters #2/#3: move D1/D2 to crossing sites
                    Dl = rpool.tile([128, L], dt.float16, tag="Dl")
                    nc.gpsimd.local_scatter(Dl[:, 0:W], D1[:, 0:W],
                                            idxs2[:, 0:W], channels=128,
                                            num_elems=W, num_idxs=W)
                    Dh = rpool.tile([128, L], dt.float16, tag="Dh")
                    nc.gpsimd.local_scatter(Dh[:, 0:W], D2[:, 0:W],
                                            idxs2[:, 0:W], channels=128,
                                            num_elems=W, num_idxs=W)

                    # G/H via reversed scans with per-partition anchors
                    G = rpool.tile([128, L], dt.bfloat16, tag="G")
                    nc.vector.tensor_tensor_scan(
                        G[:, 0:W][:, ::-1], Dl[:, 0:W][:, ::-1],
                        Dl[:, 0:W][:, ::-1], f0[:, 0:1], Alu.add, Alu.bypass)
                    H = rpool.tile([128, L], dt.bfloat16, tag="H")
                    nc.vector.tensor_tensor_scan(
                        H[:, 0:W][:, ::-1], Dh[:, 0:W][:, ::-1],
                        Dh[:, 0:W][:, ::-1], f0[:, 1:2], Alu.add, Alu.bypass)

                    # attn logits + exp + PV
                    e = rpool.tile([128, L], dt.float32, tag="e")
                    for c0 in range(0, W, 512):
                        cw = min(512, W - c0)
                        qkp = pbig.tile([128, cw], dt.float32, tag="mmps")
                        nc.tensor.matmul(qkp[:], qTt, kT[:, c0:c0 + cw],
                                         start=True, stop=True)
                        c1 = rpool.tile([128, cw], dt.bfloat16, tag="c1")
                        nc.vector.tensor_tensor(c1[:], w[:, c0:c0 + cw],
                                                H[:, c0:c0 + cw], Alu.mult)
                        c2 = rpool.tile([128, cw], dt.bfloat16, tag="c2")
                        nc.vector.tensor_tensor(c2[:], c1[:], G[:, c0:c0 + cw],
                                                Alu.add)
                        ein = rpool.tile([128, cw], dt.float32, tag="ein")
                        nc.vector.tensor_tensor(ein[:], c2[:], qkp[:], Alu.add)
                        nc.scalar.activation(e[:, c0:c0 + cw], ein[:], Act.Exp,
                                             scale=SCALE)
                    # mask invalid region of diagonal block
                    nc.vector.tensor_tensor(e[:, W - 128:W], e[:, W - 128:W],
                                            tril[:], Alu.mult)

                    acc = pacc.tile([128, D + 1], dt.float32, tag="acc")
                    for mb in range(t + 1):
                        etp = ptr.tile([128, 128], dt.float32, tag="tps")
                        nc.tensor.transpose(etp[:],
                                            e[:, mb * 128:(mb + 1) * 128],
                                            ident[:])
                        eT = epool.tile([128, 128], dt.float16, tag="eT")
                        nc.scalar.activation(eT[:], etp[:], Act.Copy)
                        nc.tensor.matmul(acc[:], eT[:], vext[mb][:],
                                         start=(mb == 0), stop=(mb == t))
                    rz = spool.tile([128, 1], dt.float32, tag="rz")
                    nc.vector.reciprocal(rz[:], acc[:, D:D + 1])
                    osb = spool.tile([128, D], dt.float16, tag="osb")
                    nc.vector.tensor_scalar(osb[:], acc[:, 0:D], rz[:], None,
                                            Alu.mult)
                    nc.sync.dma_start(out_d[b, l0:l0 + 128, :], osb[:])

    nc.compile()
    return nc


def _make_runner(nc):
    """Build a cached jitted shard_map executor for the compiled Bass module
    (mirrors concourse.bass2jax.run_bass_via_pjrt, but reusable across calls
    so per-call jax retracing/recompile cost is paid once)."""
    import jax
    import numpy as np
    from jax.sharding import Mesh, PartitionSpec
    from jax.experimental.shard_map import shard_map
    from concourse import bass2jax, mybir

    bass2jax.install_neuronx_cc_hook()
    assert nc.dbg_addr is None

    partition_name = (nc.partition_id_tensor.name
                      if nc.partition_id_tensor else None)
    in_names, out_names, out_avals = [], [], []
    for alloc in nc.m.functions[0].allocations:
        if not isinstance(alloc, mybir.MemoryLocationSet):
            continue
        name = alloc.memorylocations[0].name
        if alloc.kind == "ExternalInput":
            if name != partition_name:
                in_names.append(name)
        elif alloc.kind == "ExternalOutput":
            out_names.append(name)
            out_avals.append(jax.core.ShapedArray(
                tuple(alloc.tensor_shape), mybir.dt.np(alloc.dtype)))
    n_params = len(in_names)
    n_outs = len(out_avals)
    all_names = in_names + out_names
    if partition_name is not None:
        all_names.append(partition_name)

    def _body(*args):
        operands = list(args)
        if partition_name is not None:
            operands.append(bass2jax.partition_id_tensor())
        outs = bass2jax._bass_exec_p.bind(
            *operands,
            out_avals=tuple(out_avals),
            in_names=tuple(all_names),
            out_names=tuple(out_names),
            lowering_input_output_aliases=(),
            sim_require_finite=True,
            sim_require_nnan=True,
            nc=nc,
        )
        return tuple(outs)

    devices = jax.devices()[:N_CORES]
    mesh = Mesh(np.asarray(devices), ("core",))
    in_specs = (PartitionSpec("core"),) * (n_params + n_outs)
    out_specs = (PartitionSpec("core"),) * n_outs
    sharded = jax.jit(
        shard_map(_body, mesh=mesh, in_specs=in_specs, out_specs=out_specs,
                  check_rep=False),
        keep_unused=True)
    from jax.sharding import NamedSharding
    put_sharding = NamedSharding(mesh, PartitionSpec("core"))
    zero_shapes = [(N_CORES * a.shape[0], *a.shape[1:]) for a in out_avals]
    zero_dtypes = [a.dtype for a in out_avals]
    return sharded, in_names, put_sharding, zero_shapes, zero_dtypes


def kernel(**inputs):
    import jax

    if "nc" not in _CACHE:
        _CACHE["nc"] = _build_nc()
        _CACHE["runner"] = _make_runner(_CACHE["nc"])
        _CACHE["dev"] = {}
    sharded, in_names, put_sharding, zero_shapes, zero_dtypes = _CACHE["runner"]
    if "zeros_dev" not in _CACHE:
        import jax as _jax
        _CACHE["zeros_dev"] = [
            _jax.device_put(np.zeros(s, dtype=dd), put_sharding)
            for s, dd in zip(zero_shapes, zero_dtypes)]

    full = {
        "q": np.ascontiguousarray(inputs["query"], dtype=np.float32),
        "k": np.ascontiguousarray(inputs["key"], dtype=np.float32),
        "kc": np.ascontiguousarray(inputs["key_cope"], dtype=np.float32),
        "v": np.ascontiguousarray(inputs["val"], dtype=np.float32),
        "pe": np.tile(np.asarray(inputs["pos_emb"][0], dtype=np.float32),
                      (N_CORES, 1)),
    }
    concat_in = []
    for name in in_names:
        arr = full[name]
        ent = _CACHE["dev"].get(name)
        if ent is not None and (ent[0] is arr or np.array_equal(ent[0], arr)):
            concat_in.append(ent[1])
        else:
            da = jax.device_put(arr, put_sharding)
            _CACHE["dev"][name] = (arr, da)
            concat_in.append(da)
    out_arrs = sharded(*concat_in, *_CACHE["zeros_dev"])
    return np.asarray(out_arrs[0]).astype(np.float32)


if __name__ == "__main__":
    d = np.load("/root/problem/inputs.npz")
    out = kernel(**{kk: d[kk] for kk in d.files})
    exp = np.load("/root/problem/expected_np.npy")
    err = np.linalg.norm(out - exp) / np.linalg.norm(exp)
    print("rel err:", err)


# revision 3
# speedup vs baseline: 1.1181x; 1.1181x over previous
"""ContextPosSelfAttn (CoPE attention) — fully on-device Trainium2 Bass kernel.

Sharding: leading B (=64) dim split across 8 NeuronCores (8 batches each),
pos_emb replicated.

The data-dependent take_along_axis gather at the heart of CoPE is
restructured using the monotone-staircase property of the positions:
pos[l, m] = sum_{m'=m..l} gates[l, m'] is strictly decreasing in m with
steps < 1, so floor(pos) is a unit-step staircase. The gather of
plf[l, floor(pos[l,m])] therefore changes only at integer crossings, each
integer is crossed at most once per row, and the gathered row equals
  plf[l, 0] + reversed-cumsum over m of (crossing ? D1[l, p*] : 0)
where D1 = diff(plf). Crossing deltas are materialized with GPSIMD
local_scatter (per-partition unique indices), and the reversed cumsums run
on the DVE hardware prefix-scan (tensor_tensor_scan) with reversed access
patterns. Interpolation weight w = frac(pos) and the ceil-side slope use a
second scatter of D2 = diff(D1) through the same crossing map.
"""

import numpy as np

B, L, D = 64, 1024, 64
NPOS = 1025
N_CORES = 8
BPC = B // N_CORES
RT = L // 128          # 8 row tiles
SCALE = 0.125
PAX = NPOS + 1         # 1026, padded p-axis for scatters

_CACHE = {}


def _build_nc():
    import concourse.bacc as bacc
    import concourse.mybir as mybir
    from concourse import tile

    dt = mybir.dt
    Alu = mybir.AluOpType
    Act = mybir.ActivationFunctionType

    nc = bacc.Bacc(None, target_bir_lowering=False, debug=False)

    q_d = nc.dram_tensor("q", [BPC, L, D], dt.float32, kind="ExternalInput")
    k_d = nc.dram_tensor("k", [BPC, L, D], dt.float32, kind="ExternalInput")
    kc_d = nc.dram_tensor("kc", [BPC, L, D], dt.float32, kind="ExternalInput")
    v_d = nc.dram_tensor("v", [BPC, L, D], dt.float32, kind="ExternalInput")
    pe_d = nc.dram_tensor("pe", [D, NPOS], dt.float32, kind="ExternalInput")
    out_d = nc.dram_tensor("out", [BPC, L, D], dt.float16, kind="ExternalOutput")

    with tile.TileContext(nc) as tc:
        with (
            tc.tile_pool(name="const", bufs=1) as cpool,
            tc.tile_pool(name="inp", bufs=2) as ipool,          # per-batch inputs
            tc.tile_pool(name="row", bufs=2) as rpool,          # per-l-tile [128, *]
            tc.tile_pool(name="pax", bufs=2) as ppool,          # p-axis tensors
            tc.tile_pool(name="sm", bufs=3) as spool,           # small tiles
            tc.tile_pool(name="stg", bufs=2) as gpool,          # staging
            tc.tile_pool(name="et", bufs=3) as epool,           # transposed exp
            tc.tile_pool(name="ps", bufs=3, space="PSUM") as pbig,     # [128,512]
            tc.tile_pool(name="pst", bufs=2, space="PSUM") as ptr,     # [128,128]
            tc.tile_pool(name="psa", bufs=2, space="PSUM") as pacc,    # [128,65]
        ):
            # ---------------- constants ----------------
            ones128 = cpool.tile([128, 128], dt.float32)
            nc.vector.memset(ones128[:], 1.0)
            ident = cpool.tile([128, 128], dt.float32)
            nc.gpsimd.affine_select(ident[:], ones128[:], [[-1, 128]],
                                    Alu.is_equal, 0.0, base=0,
                                    channel_multiplier=1)
            # tril[l, m] = 1 if m <= l  (valid region of a diagonal tile)
            tril = cpool.tile([128, 128], dt.float32)
            nc.gpsimd.affine_select(tril[:], ones128[:], [[-1, 128]],
                                    Alu.is_ge, 0.0, base=0,
                                    channel_multiplier=1)
            # iota16[:, j] = j + 2 (scatter #1 payload: encoded m)
            iota16 = cpool.tile([128, L], dt.int16)
            nc.gpsimd.iota(iota16[:], [[1, L]], base=2, channel_multiplier=0)
            # pe table [64, 1025]
            pe_sb = cpool.tile([D, NPOS], dt.float32)
            nc.sync.dma_start(pe_sb[:], pe_d[:, :])

            for b in range(BPC):
                # ---------------- load + transpose inputs ----------------
                # qT/kT/kcT: [64, 1024]
                qT = ipool.tile([D, L], dt.float32, tag="qT")
                kT = ipool.tile([D, L], dt.float32, tag="kT")
                kcT = ipool.tile([D, L], dt.float32, tag="kcT")
                for (src, dstT, tg) in ((q_d, qT, "q"), (k_d, kT, "k"),
                                        (kc_d, kcT, "kc")):
                    # one strided DMA: [1024, 64] viewed as [128, (t, 64)]
                    nat = gpool.tile([128, RT * D], dt.float32, tag="nat")
                    nc.sync.dma_start(
                        nat[:].rearrange("p (t d) -> p t d", t=RT),
                        src[b].rearrange("(t p) d -> p t d", p=128))
                    for t in range(RT):
                        tp = ptr.tile([128, 128], dt.float32, tag="tps")
                        nc.tensor.transpose(tp[0:D, :],
                                            nat[:, t * D:(t + 1) * D],
                                            ident[:])
                        nc.scalar.activation(dstT[:, t * 128:(t + 1) * 128],
                                             tp[0:D, :], Act.Copy)
                # vext tiles [128, 65] fp16 with ones column
                vnat = gpool.tile([128, RT * D], dt.float32, tag="vnat")
                nc.sync.dma_start(
                    vnat[:].rearrange("p (t d) -> p t d", t=RT),
                    v_d[b].rearrange("(t p) d -> p t d", p=128))
                vext = []
                for t in range(RT):
                    vt = ipool.tile([128, D + 1], dt.float16, tag=f"vext{t}")
                    nc.vector.tensor_copy(vt[:, 0:D], vnat[:, t * D:(t + 1) * D])
                    nc.vector.memset(vt[:, D:D + 1], 1.0)
                    vext.append(vt)

                # ---------- phase 1: gates (sigmoid table) + plf/D1/D2 ------
                gts, D1s, D2s, f0s = [], [], [], []
                for t in range(RT):
                    W = 128 * (t + 1)
                    l0 = t * 128
                    qTt = qT[:, l0:l0 + 128]

                    # plf = q @ pe : [128, 1025]
                    plf = ppool.tile([128, NPOS], dt.float32, tag="plf")
                    for c0 in range(0, NPOS, 512):
                        cw = min(512, NPOS - c0)
                        pp = pbig.tile([128, cw], dt.float32, tag="mmps")
                        nc.tensor.matmul(pp[:], qTt, pe_sb[:, c0:c0 + cw],
                                         start=True, stop=True)
                        nc.scalar.activation(plf[:, c0:c0 + cw], pp[:], Act.Copy)
                    # D1[:, j] = plf[:, j] - plf[:, j-1]; col 0 is never read
                    # (its pairing index is always invalid), so left garbage.
                    D1 = ppool.tile([128, W + 1], dt.float16, tag=f"D1_{t}")
                    nc.vector.tensor_tensor(D1[:, 1:W + 1], plf[:, 1:W + 1],
                                            plf[:, 0:W], Alu.subtract)
                    # D2[:, j] = D1[:, j+1] - D1[:, j]; cols 0 / >=1024 unread
                    D2 = ppool.tile([128, W], dt.float16, tag=f"D2_{t}")
                    nc.vector.tensor_tensor(D2[:, 1:W], D1[:, 2:W + 1],
                                            D1[:, 1:W], Alu.subtract)
                    # anchors: col0 = plf[:,0] (F0), col1 = plf[:,1]-plf[:,0]
                    f0 = spool.tile([128, 2], dt.float32, tag=f"f0_{t}")
                    nc.vector.tensor_copy(f0[:, 0:1], plf[:, 0:1])
                    nc.vector.tensor_tensor(f0[:, 1:2], plf[:, 1:2],
                                            plf[:, 0:1], Alu.subtract)
                    D1s.append(D1); D2s.append(D2); f0s.append(f0)

                    # gates
                    g = rpool.tile([128, W], dt.float32, tag=f"g_{t}")
                    for c0 in range(0, W, 512):
                        cw = min(512, W - c0)
                        gp = pbig.tile([128, cw], dt.float32, tag="mmps")
                        nc.tensor.matmul(gp[:], qTt, kcT[:, c0:c0 + cw],
                                         start=True, stop=True)
                        nc.scalar.activation(g[:, c0:c0 + cw], gp[:],
                                             Act.Sigmoid, scale=SCALE)
                    nc.vector.tensor_tensor(g[:, W - 128:W], g[:, W - 128:W],
                                            tril[:], Alu.mult)
                    gts.append(g)

                # ---------- phase 2: positions/scatters/exp (exp table) -----
                oall = gpool.tile([128, RT * D], dt.float16, tag="oall")
                for t in range(RT):
                    W = 128 * (t + 1)
                    l0 = t * 128
                    qTt = qT[:, l0:l0 + 128]
                    g, D1, D2, f0 = gts[t], D1s[t], D2s[t], f0s[t]

                    # positions: reversed inclusive cumsum of g
                    pos = rpool.tile([128, L], dt.float32, tag="pos")
                    nc.vector.tensor_tensor_scan(
                        pos[:, 0:W][:, ::-1], g[:, 0:W][:, ::-1],
                        g[:, 0:W][:, ::-1], 0.0, Alu.add, Alu.bypass)

                    # fl = floor(pos) (cast rounds; pos-0.5 rounds to floor)
                    i16p = rpool.tile([128, L], dt.int16, tag="i16p")
                    nc.vector.tensor_scalar(i16p[:, 0:W], pos[:, 0:W], -0.5,
                                            None, Alu.add)
                    fl = rpool.tile([128, L + 1], dt.float16, tag="fl")
                    nc.vector.memset(fl[:, W:W + 1], 0.0)
                    nc.vector.tensor_copy(fl[:, 0:W], i16p[:, 0:W])
                    w = rpool.tile([128, L], dt.float16, tag="w")
                    nc.vector.tensor_tensor(w[:, 0:W], pos[:, 0:W], fl[:, 0:W],
                                            Alu.subtract)
                    # cross[m] = fl[m] - fl[m+1]  (0 or 1, exact)
                    cross = rpool.tile([128, L], dt.float16, tag="cross")
                    nc.vector.tensor_tensor(cross[:, 0:W], fl[:, 0:W],
                                            fl[:, 1:W + 1], Alu.subtract)
                    # idxs1 = cross * (fl + 1) - 1 -> int16 (in-place on cross)
                    nc.vector.scalar_tensor_tensor(cross[:, 0:W], fl[:, 0:W],
                                                   1.0, cross[:, 0:W],
                                                   Alu.add, Alu.mult)
                    idxs1 = rpool.tile([128, L], dt.int16, tag="idxs1")
                    nc.vector.tensor_scalar(idxs1[:, 0:W], cross[:, 0:W], -1.0,
                                            None, Alu.add)

                    # scatter #1: crossing map mu over p-axis
                    mu = ppool.tile([128, PAX], dt.int16, tag="mu")
                    nc.gpsimd.local_scatter(mu[:], iota16[:, 0:W],
                                            idxs1[:, 0:W], channels=128,
                                            num_elems=PAX, num_idxs=W)
                    idxs2 = ppool.tile([128, PAX], dt.int16, tag="idxs2")
                    nc.vector.tensor_scalar(idxs2[:], mu[:], -2, None, Alu.add)

                    # scatters #2/#3: move D1/D2 to crossing sites
                    Dl = rpool.tile([128, L], dt.float16, tag="Dl")
                    nc.gpsimd.local_scatter(Dl[:, 0:W], D1[:, 0:W],
                                            idxs2[:, 0:W], channels=128,
                                            num_elems=W, num_idxs=W)
                    Dh = rpool.tile([128, L], dt.float16, tag="Dh")
                    nc.gpsimd.local_scatter(Dh[:, 0:W], D2[:, 0:W],
                                            idxs2[:, 0:W], channels=128,
                                            num_elems=W, num_idxs=W)

                    # G/H via reversed scans with per-partition anchors
                    G = rpool.tile([128, L], dt.bfloat16, tag="G")
                    nc.vector.tensor_tensor_scan(
                        G[:, 0:W][:, ::-1], Dl[:, 0:W][:, ::-1],
                        Dl[:, 0:W][:, ::-1], f0[:, 0:1], Alu.add, Alu.bypass)
                    H = rpool.tile([128, L], dt.bfloat16, tag="H")
                    nc.vector.tensor_tensor_scan(
                        H[:, 0:W][:, ::-1], Dh[:, 0:W][:, ::-1],
                        Dh[:, 0:W][:, ::-1], f0[:, 1:2], Alu.add, Alu.bypass)

                    # attn logits + exp + PV
                    e = rpool.tile([128, L], dt.float32, tag="e")
                    for c0 in range(0, W, 512):
                        cw = min(512, W - c0)
                        qkp = pbig.tile([128, cw], dt.float32, tag="mmps")
                        nc.tensor.matmul(qkp[:], qTt, kT[:, c0:c0 + cw],
                                         start=True, stop=True)
                        c1 = rpool.tile([128, cw], dt.bfloat16, tag="c1")
                        nc.vector.tensor_tensor(c1[:], w[:, c0:c0 + cw],
                                                H[:, c0:c0 + cw], Alu.mult)
                        c2 = rpool.tile([128, cw], dt.bfloat16, tag="c2")
                        nc.vector.tensor_tensor(c2[:], c1[:], G[:, c0:c0 + cw],
                                                Alu.add)
                        ein = rpool.tile([128, cw], dt.float32, tag="ein")
                        nc.vector.tensor_tensor(ein[:], c2[:], qkp[:], Alu.add)
                        nc.scalar.activation(e[:, c0:c0 + cw], ein[:], Act.Exp,
                                             scale=SCALE)
                    # mask invalid region of diagonal block
                    nc.vector.tensor_tensor(e[:, W - 128:W], e[:, W - 128:W],
                                            tril[:], Alu.mult)

                    acc = pacc.tile([128, D + 1], dt.float32, tag="acc")
                    for mb in range(t + 1):
                        etp = ptr.tile([128, 128], dt.float32, tag="tps")
                        nc.tensor.transpose(etp[:],
                                            e[:, mb * 128:(mb + 1) * 128],
                                            ident[:])
                        eT = epool.tile([128, 128], dt.float16, tag="eT")
                        nc.scalar.activation(eT[:], etp[:], Act.Copy)
                        nc.tensor.matmul(acc[:], eT[:], vext[mb][:],
                                         start=(mb == 0), stop=(mb == t))
                    rz = spool.tile([128, 1], dt.float32, tag="rz")
                    nc.vector.reciprocal(rz[:], acc[:, D:D + 1])
                    nc.vector.tensor_scalar(oall[:, t * D:(t + 1) * D],
                                            acc[:, 0:D], rz[:], None, Alu.mult)
                if True:
                    nc.sync.dma_start(
                        out_d[b].rearrange("(t p) d -> p t d", p=128),
                        oall[:].rearrange("p (t d) -> p t d", t=RT))

    nc.compile()
    return nc


def _make_runner(nc):
    """Build a cached jitted shard_map executor for the compiled Bass module
    (mirrors concourse.bass2jax.run_bass_via_pjrt, but reusable across calls
    so per-call jax retracing/recompile cost is paid once)."""
    import jax
    import numpy as np
    from jax.sharding import Mesh, PartitionSpec
    from jax.experimental.shard_map import shard_map
    from concourse import bass2jax, mybir

    bass2jax.install_neuronx_cc_hook()
    assert nc.dbg_addr is None

    partition_name = (nc.partition_id_tensor.name
                      if nc.partition_id_tensor else None)
    in_names, out_names, out_avals = [], [], []
    for alloc in nc.m.functions[0].allocations:
        if not isinstance(alloc, mybir.MemoryLocationSet):
            continue
        name = alloc.memorylocations[0].name
        if alloc.kind == "ExternalInput":
            if name != partition_name:
                in_names.append(name)
        elif alloc.kind == "ExternalOutput":
            out_names.append(name)
            out_avals.append(jax.core.ShapedArray(
                tuple(alloc.tensor_shape), mybir.dt.np(alloc.dtype)))
    n_params = len(in_names)
    n_outs = len(out_avals)
    all_names = in_names + out_names
    if partition_name is not None:
        all_names.append(partition_name)

    def _body(*args):
        operands = list(args)
        if partition_name is not None:
            operands.append(bass2jax.partition_id_tensor())
        outs = bass2jax._bass_exec_p.bind(
            *operands,
            out_avals=tuple(out_avals),
            in_names=tuple(all_names),
            out_names=tuple(out_names),
            lowering_input_output_aliases=(),
            sim_require_finite=True,
            sim_require_nnan=True,
            nc=nc,
        )
        return tuple(outs)

    devices = jax.devices()[:N_CORES]
    mesh = Mesh(np.asarray(devices), ("core",))
    in_specs = (PartitionSpec("core"),) * (n_params + n_outs)
    out_specs = (PartitionSpec("core"),) * n_outs
    sharded = jax.jit(
        shard_map(_body, mesh=mesh, in_specs=in_specs, out_specs=out_specs,
                  check_rep=False),
        keep_unused=True)
    from jax.sharding import NamedSharding
    put_sharding = NamedSharding(mesh, PartitionSpec("core"))
    zero_shapes = [(N_CORES * a.shape[0], *a.shape[1:]) for a in out_avals]
    zero_dtypes = [a.dtype for a in out_avals]
    return sharded, in_names, put_sharding, zero_shapes, zero_dtypes


def kernel(**inputs):
    import jax

    if "nc" not in _CACHE:
        _CACHE["nc"] = _build_nc()
        _CACHE["runner"] = _make_runner(_CACHE["nc"])
        _CACHE["dev"] = {}
    sharded, in_names, put_sharding, zero_shapes, zero_dtypes = _CACHE["runner"]
    if "zeros_dev" not in _CACHE:
        import jax as _jax
        _CACHE["zeros_dev"] = [
            _jax.device_put(np.zeros(s, dtype=dd), put_sharding)
            for s, dd in zip(zero_shapes, zero_dtypes)]

    full = {
        "q": np.ascontiguousarray(inputs["query"], dtype=np.float32),
        "k": np.ascontiguousarray(inputs["key"], dtype=np.float32),
        "kc": np.ascontiguousarray(inputs["key_cope"], dtype=np.float32),
        "v": np.ascontiguousarray(inputs["val"], dtype=np.float32),
        "pe": np.tile(np.asarray(inputs["pos_emb"][0], dtype=np.float32),
                      (N_CORES, 1)),
    }
    concat_in = []
    for name in in_names:
        arr = full[name]
        ent = _CACHE["dev"].get(name)
        if ent is not None and (ent[0] is arr or np.array_equal(ent[0], arr)):
            concat_in.append(ent[1])
        else:
            da = jax.device_put(arr, put_sharding)
            _CACHE["dev"][name] = (arr, da)
            concat_in.append(da)
    out_arrs = sharded(*concat_in, *_CACHE["zeros_dev"])
    return np.asarray(out_arrs[0]).astype(np.float32)


if __name__ == "__main__":
    d = np.load("/root/problem/inputs.npz")
    out = kernel(**{kk: d[kk] for kk in d.files})
    exp = np.load("/root/problem/expected_np.npy")
    err = np.linalg.norm(out - exp) / np.linalg.norm(exp)
    print("rel err:", err)


# revision 4
# speedup vs baseline: 1.1244x; 1.0056x over previous
"""ContextPosSelfAttn (CoPE attention) — fully on-device Trainium2 Bass kernel.

Sharding: leading B (=64) dim split across 8 NeuronCores (8 batches each),
pos_emb replicated.

The data-dependent take_along_axis gather at the heart of CoPE is
restructured using the monotone-staircase property of the positions:
pos[l, m] = sum_{m'=m..l} gates[l, m'] is strictly decreasing in m with
steps < 1, so floor(pos) is a unit-step staircase. The gather of
plf[l, floor(pos[l,m])] therefore changes only at integer crossings, each
integer is crossed at most once per row, and the gathered row equals
  plf[l, 0] + reversed-cumsum over m of (crossing ? D1[l, p*] : 0)
where D1 = diff(plf). Crossing deltas are materialized with GPSIMD
local_scatter (per-partition unique indices), and the reversed cumsums run
on the DVE hardware prefix-scan (tensor_tensor_scan) with reversed access
patterns. Interpolation weight w = frac(pos) and the ceil-side slope use a
second scatter of D2 = diff(D1) through the same crossing map.
"""

import numpy as np

B, L, D = 64, 1024, 64
NPOS = 1025
N_CORES = 8
BPC = B // N_CORES
RT = L // 128          # 8 row tiles
SCALE = 0.125
PAX = NPOS + 1         # 1026, padded p-axis for scatters

_CACHE = {}


def _build_nc():
    import concourse.bacc as bacc
    import concourse.mybir as mybir
    from concourse import tile

    dt = mybir.dt
    Alu = mybir.AluOpType
    Act = mybir.ActivationFunctionType

    nc = bacc.Bacc(None, target_bir_lowering=False, debug=False)

    q_d = nc.dram_tensor("q", [BPC, L, D], dt.float32, kind="ExternalInput")
    k_d = nc.dram_tensor("k", [BPC, L, D], dt.float32, kind="ExternalInput")
    kc_d = nc.dram_tensor("kc", [BPC, L, D], dt.float32, kind="ExternalInput")
    v_d = nc.dram_tensor("v", [BPC, L, D], dt.float32, kind="ExternalInput")
    pe_d = nc.dram_tensor("pe", [D, NPOS], dt.float32, kind="ExternalInput")
    out_d = nc.dram_tensor("out", [BPC, L, D], dt.float16, kind="ExternalOutput")

    with tile.TileContext(nc) as tc:
        with (
            tc.tile_pool(name="const", bufs=1) as cpool,
            tc.tile_pool(name="inp", bufs=2) as ipool,          # per-batch inputs
            tc.tile_pool(name="row", bufs=2) as rpool,          # per-l-tile [128, *]
            tc.tile_pool(name="pax", bufs=2) as ppool,          # p-axis tensors
            tc.tile_pool(name="sm", bufs=3) as spool,           # small tiles
            tc.tile_pool(name="stg", bufs=2) as gpool,          # staging
            tc.tile_pool(name="et", bufs=3) as epool,           # transposed exp
            tc.tile_pool(name="ps", bufs=3, space="PSUM") as pbig,     # [128,512]
            tc.tile_pool(name="pst", bufs=2, space="PSUM") as ptr,     # [128,128]
            tc.tile_pool(name="psa", bufs=2, space="PSUM") as pacc,    # [128,65]
        ):
            # ---------------- constants ----------------
            ones128 = cpool.tile([128, 128], dt.float32)
            nc.vector.memset(ones128[:], 1.0)
            ident = cpool.tile([128, 128], dt.float32)
            nc.gpsimd.affine_select(ident[:], ones128[:], [[-1, 128]],
                                    Alu.is_equal, 0.0, base=0,
                                    channel_multiplier=1)
            # tril[l, m] = 1 if m <= l  (valid region of a diagonal tile)
            tril = cpool.tile([128, 128], dt.float32)
            nc.gpsimd.affine_select(tril[:], ones128[:], [[-1, 128]],
                                    Alu.is_ge, 0.0, base=0,
                                    channel_multiplier=1)
            # iota16[:, j] = j + 2 (scatter #1 payload: encoded m)
            iota16 = cpool.tile([128, L], dt.int16)
            nc.gpsimd.iota(iota16[:], [[1, L]], base=2, channel_multiplier=0)
            # pe table [64, 1025]
            pe_sb = cpool.tile([D, NPOS], dt.float32)
            nc.sync.dma_start(pe_sb[:], pe_d[:, :])

            for b in range(BPC):
                # ---------------- load + transpose inputs ----------------
                # qT/kT/kcT: [64, 1024]
                qT = ipool.tile([D, L], dt.float32, tag="qT")
                kT = ipool.tile([D, L], dt.float32, tag="kT")
                kcT = ipool.tile([D, L], dt.float32, tag="kcT")
                for (src, dstT, tg) in ((q_d, qT, "q"), (k_d, kT, "k"),
                                        (kc_d, kcT, "kc")):
                    # one strided DMA: [1024, 64] viewed as [128, (t, 64)]
                    nat = gpool.tile([128, RT * D], dt.float32, tag="nat")
                    nc.sync.dma_start(
                        nat[:].rearrange("p (t d) -> p t d", t=RT),
                        src[b].rearrange("(t p) d -> p t d", p=128))
                    for t in range(RT):
                        tp = ptr.tile([128, 128], dt.float32, tag="tps")
                        nc.tensor.transpose(tp[0:D, :],
                                            nat[:, t * D:(t + 1) * D],
                                            ident[:])
                        nc.scalar.activation(dstT[:, t * 128:(t + 1) * 128],
                                             tp[0:D, :], Act.Copy)
                # vext tiles [128, 65] fp16 with ones column
                vnat = gpool.tile([128, RT * D], dt.float32, tag="vnat")
                nc.sync.dma_start(
                    vnat[:].rearrange("p (t d) -> p t d", t=RT),
                    v_d[b].rearrange("(t p) d -> p t d", p=128))
                vext = []
                for t in range(RT):
                    vt = ipool.tile([128, D + 1], dt.float16, tag=f"vext{t}")
                    nc.vector.tensor_copy(vt[:, 0:D], vnat[:, t * D:(t + 1) * D])
                    nc.vector.memset(vt[:, D:D + 1], 1.0)
                    vext.append(vt)

                # ---------- phase 1: gates (sigmoid table) + plf/D1/D2 ------
                gts, D1s, D2s, f0s = [], [], [], []
                for t in range(RT):
                    W = 128 * (t + 1)
                    l0 = t * 128
                    qTt = qT[:, l0:l0 + 128]

                    # plf = q @ pe, only columns this tile can index
                    # (floor(pos)+1 <= W+1), fp16 to halve downstream traffic
                    PW = min(W + 2, NPOS)
                    plf = ppool.tile([128, PW], dt.float16, tag="plf")
                    for c0 in range(0, PW, 512):
                        cw = min(512, PW - c0)
                        pp = pbig.tile([128, cw], dt.float32, tag="mmps")
                        nc.tensor.matmul(pp[:], qTt, pe_sb[:, c0:c0 + cw],
                                         start=True, stop=True)
                        nc.scalar.activation(plf[:, c0:c0 + cw], pp[:], Act.Copy)
                    # D1[:, j] = plf[:, j] - plf[:, j-1]; col 0 is never read
                    # (its pairing index is always invalid), so left garbage.
                    D1 = ppool.tile([128, W + 1], dt.float16, tag=f"D1_{t}")
                    nc.vector.tensor_tensor(D1[:, 1:W + 1], plf[:, 1:W + 1],
                                            plf[:, 0:W], Alu.subtract)
                    # D2[:, j] = D1[:, j+1] - D1[:, j]; cols 0 / >=1024 unread
                    D2 = ppool.tile([128, W], dt.float16, tag=f"D2_{t}")
                    nc.vector.tensor_tensor(D2[:, 1:W], D1[:, 2:W + 1],
                                            D1[:, 1:W], Alu.subtract)
                    # anchors: col0 = plf[:,0] (F0), col1 = plf[:,1]-plf[:,0]
                    f0 = spool.tile([128, 2], dt.float32, tag=f"f0_{t}")
                    nc.vector.tensor_copy(f0[:, 0:1], plf[:, 0:1])
                    nc.vector.tensor_tensor(f0[:, 1:2], plf[:, 1:2],
                                            plf[:, 0:1], Alu.subtract)
                    D1s.append(D1); D2s.append(D2); f0s.append(f0)

                    # gates
                    g = rpool.tile([128, W], dt.float32, tag=f"g_{t}")
                    for c0 in range(0, W, 512):
                        cw = min(512, W - c0)
                        gp = pbig.tile([128, cw], dt.float32, tag="mmps")
                        nc.tensor.matmul(gp[:], qTt, kcT[:, c0:c0 + cw],
                                         start=True, stop=True)
                        nc.scalar.activation(g[:, c0:c0 + cw], gp[:],
                                             Act.Sigmoid, scale=SCALE)
                    nc.vector.tensor_tensor(g[:, W - 128:W], g[:, W - 128:W],
                                            tril[:], Alu.mult)
                    gts.append(g)

                # ---------- phase 2: positions/scatters/exp (exp table) -----
                oall = gpool.tile([128, RT * D], dt.float16, tag="oall")
                for t in range(RT):
                    W = 128 * (t + 1)
                    l0 = t * 128
                    qTt = qT[:, l0:l0 + 128]
                    g, D1, D2, f0 = gts[t], D1s[t], D2s[t], f0s[t]

                    # positions: reversed inclusive cumsum of g
                    pos = rpool.tile([128, L], dt.float32, tag="pos")
                    nc.vector.tensor_tensor_scan(
                        pos[:, 0:W][:, ::-1], g[:, 0:W][:, ::-1],
                        g[:, 0:W][:, ::-1], 0.0, Alu.add, Alu.bypass)

                    # fl = floor(pos) (cast rounds; pos-0.5 rounds to floor)
                    i16p = rpool.tile([128, L], dt.int16, tag="i16p")
                    nc.vector.tensor_scalar(i16p[:, 0:W], pos[:, 0:W], -0.5,
                                            None, Alu.add)
                    fl = rpool.tile([128, L + 1], dt.float16, tag="fl")
                    nc.vector.memset(fl[:, W:W + 1], 0.0)
                    nc.vector.tensor_copy(fl[:, 0:W], i16p[:, 0:W])
                    w = rpool.tile([128, L], dt.float16, tag="w")
                    nc.vector.tensor_tensor(w[:, 0:W], pos[:, 0:W], fl[:, 0:W],
                                            Alu.subtract)
                    # cross[m] = fl[m] - fl[m+1]  (0 or 1, exact)
                    cross = rpool.tile([128, L], dt.float16, tag="cross")
                    nc.vector.tensor_tensor(cross[:, 0:W], fl[:, 0:W],
                                            fl[:, 1:W + 1], Alu.subtract)
                    # idxs1 = cross * (fl + 1) - 1 -> int16 (in-place on cross)
                    nc.vector.scalar_tensor_tensor(cross[:, 0:W], fl[:, 0:W],
                                                   1.0, cross[:, 0:W],
                                                   Alu.add, Alu.mult)
                    idxs1 = rpool.tile([128, L], dt.int16, tag="idxs1")
                    nc.vector.tensor_scalar(idxs1[:, 0:W], cross[:, 0:W], -1.0,
                                            None, Alu.add)

                    # scatter #1: crossing map mu over p-axis
                    mu = ppool.tile([128, PAX], dt.int16, tag="mu")
                    nc.gpsimd.local_scatter(mu[:], iota16[:, 0:W],
                                            idxs1[:, 0:W], channels=128,
                                            num_elems=PAX, num_idxs=W)
                    idxs2 = ppool.tile([128, PAX], dt.int16, tag="idxs2")
                    nc.vector.tensor_scalar(idxs2[:, 0:W], mu[:, 0:W], -2,
                                            None, Alu.add)

                    # scatters #2/#3: move D1/D2 to crossing sites
                    Dl = rpool.tile([128, L], dt.float16, tag="Dl")
                    nc.gpsimd.local_scatter(Dl[:, 0:W], D1[:, 0:W],
                                            idxs2[:, 0:W], channels=128,
                                            num_elems=W, num_idxs=W)
                    Dh = rpool.tile([128, L], dt.float16, tag="Dh")
                    nc.gpsimd.local_scatter(Dh[:, 0:W], D2[:, 0:W],
                                            idxs2[:, 0:W], channels=128,
                                            num_elems=W, num_idxs=W)

                    # G/H via reversed scans with per-partition anchors
                    G = rpool.tile([128, L], dt.bfloat16, tag="G")
                    nc.vector.tensor_tensor_scan(
                        G[:, 0:W][:, ::-1], Dl[:, 0:W][:, ::-1],
                        Dl[:, 0:W][:, ::-1], f0[:, 0:1], Alu.add, Alu.bypass)
                    H = rpool.tile([128, L], dt.bfloat16, tag="H")
                    nc.vector.tensor_tensor_scan(
                        H[:, 0:W][:, ::-1], Dh[:, 0:W][:, ::-1],
                        Dh[:, 0:W][:, ::-1], f0[:, 1:2], Alu.add, Alu.bypass)

                    # attn logits + exp + PV
                    e = rpool.tile([128, L], dt.float32, tag="e")
                    for c0 in range(0, W, 512):
                        cw = min(512, W - c0)
                        qkp = pbig.tile([128, cw], dt.float32, tag="mmps")
                        nc.tensor.matmul(qkp[:], qTt, kT[:, c0:c0 + cw],
                                         start=True, stop=True)
                        c1 = rpool.tile([128, cw], dt.bfloat16, tag="c1")
                        nc.vector.tensor_tensor(c1[:], w[:, c0:c0 + cw],
                                                H[:, c0:c0 + cw], Alu.mult)
                        c2 = rpool.tile([128, cw], dt.bfloat16, tag="c2")
                        nc.vector.tensor_tensor(c2[:], c1[:], G[:, c0:c0 + cw],
                                                Alu.add)
                        ein = rpool.tile([128, cw], dt.float32, tag="ein")
                        nc.vector.tensor_tensor(ein[:], c2[:], qkp[:], Alu.add)
                        nc.scalar.activation(e[:, c0:c0 + cw], ein[:], Act.Exp,
                                             scale=SCALE)
                    # mask invalid region of diagonal block
                    nc.vector.tensor_tensor(e[:, W - 128:W], e[:, W - 128:W],
                                            tril[:], Alu.mult)

                    acc = pacc.tile([128, D + 1], dt.float32, tag="acc")
                    for mb in range(t + 1):
                        etp = ptr.tile([128, 128], dt.float32, tag="tps")
                        nc.tensor.transpose(etp[:],
                                            e[:, mb * 128:(mb + 1) * 128],
                                            ident[:])
                        eT = epool.tile([128, 128], dt.float16, tag="eT")
                        nc.scalar.activation(eT[:], etp[:], Act.Copy)
                        nc.tensor.matmul(acc[:], eT[:], vext[mb][:],
                                         start=(mb == 0), stop=(mb == t))
                    rz = spool.tile([128, 1], dt.float32, tag="rz")
                    nc.vector.reciprocal(rz[:], acc[:, D:D + 1])
                    nc.vector.tensor_scalar(oall[:, t * D:(t + 1) * D],
                                            acc[:, 0:D], rz[:], None, Alu.mult)
                if True:
                    nc.sync.dma_start(
                        out_d[b].rearrange("(t p) d -> p t d", p=128),
                        oall[:].rearrange("p (t d) -> p t d", t=RT))

    nc.compile()
    return nc


def _make_runner(nc):
    """Build a cached jitted shard_map executor for the compiled Bass module
    (mirrors concourse.bass2jax.run_bass_via_pjrt, but reusable across calls
    so per-call jax retracing/recompile cost is paid once)."""
    import jax
    import numpy as np
    from jax.sharding import Mesh, PartitionSpec
    from jax.experimental.shard_map import shard_map
    from concourse import bass2jax, mybir

    bass2jax.install_neuronx_cc_hook()
    assert nc.dbg_addr is None

    partition_name = (nc.partition_id_tensor.name
                      if nc.partition_id_tensor else None)
    in_names, out_names, out_avals = [], [], []
    for alloc in nc.m.functions[0].allocations:
        if not isinstance(alloc, mybir.MemoryLocationSet):
            continue
        name = alloc.memorylocations[0].name
        if alloc.kind == "ExternalInput":
            if name != partition_name:
                in_names.append(name)
        elif alloc.kind == "ExternalOutput":
            out_names.append(name)
            out_avals.append(jax.core.ShapedArray(
                tuple(alloc.tensor_shape), mybir.dt.np(alloc.dtype)))
    n_params = len(in_names)
    n_outs = len(out_avals)
    all_names = in_names + out_names
    if partition_name is not None:
        all_names.append(partition_name)

    def _body(*args):
        operands = list(args)
        if partition_name is not None:
            operands.append(bass2jax.partition_id_tensor())
        outs = bass2jax._bass_exec_p.bind(
            *operands,
            out_avals=tuple(out_avals),
            in_names=tuple(all_names),
            out_names=tuple(out_names),
            lowering_input_output_aliases=(),
            sim_require_finite=True,
            sim_require_nnan=True,
            nc=nc,
        )
        return tuple(outs)

    devices = jax.devices()[:N_CORES]
    mesh = Mesh(np.asarray(devices), ("core",))
    in_specs = (PartitionSpec("core"),) * (n_params + n_outs)
    out_specs = (PartitionSpec("core"),) * n_outs
    sharded = jax.jit(
        shard_map(_body, mesh=mesh, in_specs=in_specs, out_specs=out_specs,
                  check_rep=False),
        keep_unused=True)
    from jax.sharding import NamedSharding
    put_sharding = NamedSharding(mesh, PartitionSpec("core"))
    zero_shapes = [(N_CORES * a.shape[0], *a.shape[1:]) for a in out_avals]
    zero_dtypes = [a.dtype for a in out_avals]
    return sharded, in_names, put_sharding, zero_shapes, zero_dtypes


def kernel(**inputs):
    import jax

    if "nc" not in _CACHE:
        _CACHE["nc"] = _build_nc()
        _CACHE["runner"] = _make_runner(_CACHE["nc"])
        _CACHE["dev"] = {}
    sharded, in_names, put_sharding, zero_shapes, zero_dtypes = _CACHE["runner"]
    if "zeros_dev" not in _CACHE:
        import jax as _jax
        _CACHE["zeros_dev"] = [
            _jax.device_put(np.zeros(s, dtype=dd), put_sharding)
            for s, dd in zip(zero_shapes, zero_dtypes)]

    full = {
        "q": np.ascontiguousarray(inputs["query"], dtype=np.float32),
        "k": np.ascontiguousarray(inputs["key"], dtype=np.float32),
        "kc": np.ascontiguousarray(inputs["key_cope"], dtype=np.float32),
        "v": np.ascontiguousarray(inputs["val"], dtype=np.float32),
        "pe": np.tile(np.asarray(inputs["pos_emb"][0], dtype=np.float32),
                      (N_CORES, 1)),
    }
    concat_in = []
    for name in in_names:
        arr = full[name]
        ent = _CACHE["dev"].get(name)
        if ent is not None and (ent[0] is arr or np.array_equal(ent[0], arr)):
            concat_in.append(ent[1])
        else:
            da = jax.device_put(arr, put_sharding)
            _CACHE["dev"][name] = (arr, da)
            concat_in.append(da)
    out_arrs = sharded(*concat_in, *_CACHE["zeros_dev"])
    return np.asarray(out_arrs[0]).astype(np.float32)


if __name__ == "__main__":
    d = np.load("/root/problem/inputs.npz")
    out = kernel(**{kk: d[kk] for kk in d.files})
    exp = np.load("/root/problem/expected_np.npy")
    err = np.linalg.norm(out - exp) / np.linalg.norm(exp)
    print("rel err:", err)
